# revision 25
# baseline (speedup 1.0000x reference)
import sys

sys.path.insert(0, "/opt/trn_rl_repo")

import numpy as np

import concourse.bass as bass
import concourse.mybir as mybir
import concourse.tile as tile
from concourse import bacc
from concourse.bass_utils import run_bass_kernel_spmd
from concourse.masks import make_identity

# Problem dims (hardcoded per harness contract)
N, S, C = 4096, 1, 512
E, H, V = 64, 512, 256
T_STEPS = 32
M = 8            # cores
NL = N // M      # 512 rows per core
P = 128
KH = H // P      # 4 k-tiles over hidden dim
KV = V // P      # 2 k-tiles over vocab dim
NB = NL // P     # 4 batch tiles per core

F32 = mybir.dt.float32
F32R = mybir.dt.float32r
BF16 = mybir.dt.bfloat16

_PROGRAM = None
LAST_RESULT = None


def _build_program():
    nc = bacc.Bacc("TRN2", target_bir_lowering=False, debug=False)

    whh_d = nc.dram_tensor("whh", [KH, P, 3 * H], F32R, kind="ExternalInput")
    wfold_d = nc.dram_tensor("wfold", [KV, P, 3 * H], F32R, kind="ExternalInput")
    gctx_d = nc.dram_tensor("gctx", [3 * H // P, P, NL], F32, kind="ExternalInput")
    fcwh_d = nc.dram_tensor("fcwh", [KH, P, V], F32R, kind="ExternalInput")
    fcwfold_d = nc.dram_tensor("fcwfold", [KV, P, V], F32R, kind="ExternalInput")
    lctx_d = nc.dram_tensor("lctx", [NB, P, V], F32, kind="ExternalInput")
    bhhn_d = nc.dram_tensor("bhhn", [P, KH], F32, kind="ExternalInput")
    oh0T_d = nc.dram_tensor("oh0T", [KV, P, NL], F32R, kind="ExternalInput")
    out_d = nc.dram_tensor("out", [NL, T_STEPS, V], F32, kind="ExternalOutput")

    Sig = mybir.ActivationFunctionType.Sigmoid
    Copy = mybir.ActivationFunctionType.Copy
    Tanh = mybir.ActivationFunctionType.Tanh
    ADD = mybir.AluOpType.add
    MULT = mybir.AluOpType.mult
    SUB = mybir.AluOpType.subtract
    ISEQ = mybir.AluOpType.is_equal
    MG = 3 * H // P  # 12 gate m-tiles

    with tile.TileContext(nc) as tc:
        with tc.tile_pool(name="const", bufs=1) as const, \
             tc.tile_pool(name="state", bufs=2) as state, \
             tc.tile_pool(name="gate", bufs=2) as gate, \
             tc.tile_pool(name="work", bufs=4) as work, \
             tc.tile_pool(name="outp", bufs=2) as outp, \
             tc.tile_pool(name="pg", bufs=3, space="PSUM") as pgp, \
             tc.tile_pool(name="px", bufs=2, space="PSUM") as pxp, \
             tc.tile_pool(name="pl", bufs=2, space="PSUM") as plp, \
             tc.tile_pool(name="pt", bufs=1, space="PSUM") as ptp:

            identb = const.tile([P, P], BF16)
            make_identity(nc, identb)

            # ---- constant loads (ordered so step-0 deps come first) ----
            oh0T = const.tile([P, KV, NL], F32R)
            for k in range(KV):
                nc.sync.dma_start(out=oh0T[:, k, :], in_=oh0T_d[k])
            wfold = const.tile([P, KV, 3 * H], F32R)
            for k in range(KV):
                nc.sync.dma_start(out=wfold[:, k, :], in_=wfold_d[k])
            gctx = const.tile([P, MG, NL], F32)
            for k in range(MG):
                nc.sync.dma_start(out=gctx[:, k, :], in_=gctx_d[k])
            bhhn = const.tile([P, KH], F32)
            nc.sync.dma_start(out=bhhn, in_=bhhn_d[:, :])
            fcwh = const.tile([P, KH, V], F32R)
            for k in range(KH):
                nc.sync.dma_start(out=fcwh[:, k, :], in_=fcwh_d[k])
            fcwfold = const.tile([P, KV, V], F32R)
            for k in range(KV):
                nc.sync.dma_start(out=fcwfold[:, k, :], in_=fcwfold_d[k])
            lctx = const.tile([P, NB, V], F32)
            for k in range(NB):
                nc.sync.dma_start(out=lctx[:, k, :], in_=lctx_d[k])
            whh = const.tile([P, KH, 3 * H], F32R)
            for k in range(KH):
                nc.sync.dma_start(out=whh[:, k, :], in_=whh_d[k])

            HF = NL // 2
            hT_prev = None
            oh_prev = None
            for t in range(T_STEPS):
                r_t = gate.tile([P, KH, NL], F32, tag="r")
                z_t = gate.tile([P, KH, NL], F32, tag="z")
                n_t = gate.tile([P, KH, NL], F32, tag="n")
                hT_cur = state.tile([P, KH, NL], F32R, tag="h")
                oht = oh0T if t == 0 else state.tile([P, KV, NL], F32R,
                                                     tag="oht")
                mx = work.tile([P, NB], F32, tag="mx")
                oh_nv = gate.tile([P, NB, V], BF16, tag="oh")

                for q in range(2):
                    qsl = slice(q * HF, (q + 1) * HF)
                    # ---- one-hot transpose for this half (prev step) ----
                    if t > 0:
                        pt = ptp.tile([P, NL], BF16, tag="pt")
                        for vb in range(KV):
                            for nb in range(2):
                                nc.tensor.transpose(
                                    pt[:, vb * HF + nb * P:vb * HF + (nb + 1) * P],
                                    oh_prev[:, nb + 2 * q, vb * P:(vb + 1) * P],
                                    identb)
                        for vb in range(KV):
                            nc.vector.tensor_copy(
                                oht[:, vb, qsl],
                                pt[:, vb * HF:(vb + 1) * HF])

                    # ---- r,z gates for this half ----
                    for m in range(2 * KH):
                        msl = slice(m * P, (m + 1) * P)
                        pg = pgp.tile([P, NL], F32, tag="pg")
                        pgv = pg[:, 0:HF]
                        dst = (r_t[:, m, qsl] if m < KH
                               else z_t[:, m - KH, qsl])
                        if t > 0:
                            nc.scalar.activation(pgv, gctx[:, m, qsl], Copy,
                                                 0.0, 1.0)
                            for k in range(KH):
                                nc.tensor.matmul(pgv, whh[:, k, msl],
                                                 hT_prev[:, k, qsl],
                                                 start=False, stop=False,
                                                 skip_group_check=True)
                            nc.tensor.matmul(pgv, wfold[:, 0, msl],
                                             oht[:, 0, qsl],
                                             start=False, stop=False,
                                             skip_group_check=True)
                            nc.tensor.matmul(pgv, wfold[:, 1, msl],
                                             oht[:, 1, qsl],
                                             start=False, stop=True,
                                             skip_group_check=True)
                            nc.scalar.activation(dst, pgv, Sig)
                        else:
                            nc.tensor.matmul(pgv, wfold[:, 0, msl],
                                             oht[:, 0, qsl],
                                             start=True, stop=False)
                            nc.tensor.matmul(pgv, wfold[:, 1, msl],
                                             oht[:, 1, qsl],
                                             start=False, stop=True)
                            nc.vector.tensor_add(dst, pgv, gctx[:, m, qsl])
                            nc.scalar.activation(dst, dst, Sig)

                    # ---- n gate + h update for this half ----
                    pxs = []
                    for i in range(KH):
                        m = 2 * KH + i
                        msl = slice(m * P, (m + 1) * P)
                        px = pxp.tile([P, NL], F32, tag="px")
                        pxv = px[:, 0:HF]
                        pxs.append(pxv)
                        if t > 0:
                            nc.scalar.activation(pxv, gctx[:, m, qsl],
                                                 Copy, 0.0, 1.0)
                            nc.tensor.matmul(pxv, wfold[:, 0, msl],
                                             oht[:, 0, qsl],
                                             start=False, stop=False,
                                             skip_group_check=True)
                            nc.tensor.matmul(pxv, wfold[:, 1, msl],
                                             oht[:, 1, qsl],
                                             start=False, stop=True,
                                             skip_group_check=True)
                        else:
                            nc.tensor.matmul(pxv, wfold[:, 0, msl],
                                             oht[:, 0, qsl],
                                             start=True, stop=False)
                            nc.tensor.matmul(pxv, wfold[:, 1, msl],
                                             oht[:, 1, qsl],
                                             start=False, stop=True)
                    for i in range(KH):
                        m = 2 * KH + i
                        msl = slice(m * P, (m + 1) * P)
                        u = work.tile([P, NL], F32, tag="u")
                        uv = u[:, 0:HF]
                        if t > 0:
                            pgh = pgp.tile([P, NL], F32, tag="pg")
                            pghv = pgh[:, 0:HF]
                            for k in range(KH):
                                nc.tensor.matmul(pghv, whh[:, k, msl],
                                                 hT_prev[:, k, qsl],
                                                 start=(k == 0),
                                                 stop=(k == KH - 1))
                            nc.vector.scalar_tensor_tensor(
                                uv, pghv, bhhn[:, i:i + 1], r_t[:, i, qsl],
                                ADD, MULT)
                        else:
                            nc.vector.tensor_scalar(uv, r_t[:, i, qsl],
                                                    bhhn[:, i:i + 1],
                                                    None, MULT)
                        nc.vector.tensor_add(n_t[:, i, qsl], pxs[i], uv)
                        if t == 0:
                            nc.vector.tensor_add(n_t[:, i, qsl],
                                                 n_t[:, i, qsl],
                                                 gctx[:, m, qsl])
                        nc.scalar.activation(n_t[:, i, qsl], n_t[:, i, qsl],
                                             Tanh)
                        v = work.tile([P, NL], F32, tag="v")
                        vv = v[:, 0:HF]
                        if t > 0:
                            nc.gpsimd.tensor_sub(vv, hT_prev[:, i, qsl],
                                                 n_t[:, i, qsl])
                            nc.gpsimd.tensor_mul(vv, vv, z_t[:, i, qsl])
                            nc.vector.tensor_add(hT_cur[:, i, qsl], vv,
                                                 n_t[:, i, qsl])
                        else:
                            nc.vector.tensor_scalar(vv, z_t[:, i, qsl],
                                                    -1.0, 1.0, MULT, ADD)
                            nc.vector.tensor_mul(hT_cur[:, i, qsl], vv,
                                                 n_t[:, i, qsl])

                    # ---- logits + argmax for this half ----
                    for nb in (2 * q, 2 * q + 1):
                        nsl = slice(nb * P, (nb + 1) * P)
                        pl = plp.tile([P, NL], F32, tag="pl")
                        plv = pl[:, 0:V]
                        if t > 0:
                            nc.scalar.activation(plv, lctx[:, nb, :], Copy,
                                                 0.0, 1.0)
                            nc.tensor.matmul(plv, oht[:, 0, nsl],
                                             fcwfold[:, 0, :],
                                             start=False, stop=False,
                                             skip_group_check=True)
                            nc.tensor.matmul(plv, oht[:, 1, nsl],
                                             fcwfold[:, 1, :],
                                             start=False, stop=False,
                                             skip_group_check=True)
                            for k in range(KH):
                                nc.tensor.matmul(plv, hT_cur[:, k, nsl],
                                                 fcwh[:, k, :],
                                                 start=False,
                                                 stop=(k == KH - 1),
                                                 skip_group_check=True)
                            lg = outp.tile([P, V], F32, tag="lg")
                            nc.scalar.activation(lg, plv, Copy, 0.0, 1.0)
                        else:
                            nc.tensor.matmul(plv, oht[:, 0, nsl],
                                             fcwfold[:, 0, :],
                                             start=True, stop=False)
                            nc.tensor.matmul(plv, oht[:, 1, nsl],
                                             fcwfold[:, 1, :],
                                             start=False, stop=False)
                            for k in range(KH):
                                nc.tensor.matmul(plv, hT_cur[:, k, nsl],
                                                 fcwh[:, k, :],
                                                 start=False,
                                                 stop=(k == KH - 1))
                            lg = outp.tile([P, V], F32, tag="lg")
                            nc.vector.tensor_add(lg, plv, lctx[:, nb, :])
                        nc.sync.dma_start(out=out_d[nsl, t, :], in_=lg)
                        if t < T_STEPS - 1:
                            amsrc = plv if t > 0 else lg
                            nc.vector.tensor_reduce(out=mx[:, nb:nb + 1],
                                                    in_=amsrc,
                                                    axis=mybir.AxisListType.X,
                                                    op=mybir.AluOpType.max)
                            nc.vector.tensor_scalar(oh_nv[:, nb, :], amsrc,
                                                    mx[:, nb:nb + 1], None,
                                                    ISEQ)

                oh_prev = oh_nv
                hT_prev = hT_cur

    nc.compile()
    return nc


def _get_program():
    global _PROGRAM
    if _PROGRAM is None:
        _PROGRAM = _build_program()
    return _PROGRAM


def kernel(encoded, init_token, emb_W, W_ih, W_hh, b_ih, b_hh, fc_W, fc_b, T):
    global LAST_RESULT
    assert int(T) == T_STEPS
    encoded = np.asarray(encoded, np.float64)
    init_token = np.asarray(init_token).astype(np.int64)
    emb_W = np.asarray(emb_W, np.float64)
    W_ih = np.asarray(W_ih, np.float64)
    W_hh = np.asarray(W_hh, np.float64)
    b_ih = np.asarray(b_ih, np.float64)
    b_hh = np.asarray(b_hh, np.float64)
    fc_W = np.asarray(fc_W, np.float64)
    fc_b = np.asarray(fc_b, np.float64)

    cx = np.ascontiguousarray

    # shared weights
    whh = cx(W_hh.T.reshape(KH, P, 3 * H).astype(np.float32))
    wfold = cx((W_ih[:, :E] @ emb_W.T).T.reshape(KV, P, 3 * H).astype(np.float32))
    fcwh = cx(fc_W[:, E + C:].T.reshape(KH, P, V).astype(np.float32))
    fcwfold = cx((fc_W[:, :E] @ emb_W.T).T.reshape(KV, P, V).astype(np.float32))
    bhhn = cx(b_hh[2 * H:].reshape(KH, P).T.astype(np.float32))

    # context GEMMs precomputed exactly on host (fp64)
    ctx_all = encoded.reshape(N, C)
    bias_g = b_ih.copy()
    bias_g[:2 * H] += b_hh[:2 * H]
    gctx_all = ctx_all @ W_ih[:, E:].T + bias_g          # [N, 3H]
    lctx_all = ctx_all @ fc_W[:, E:E + C].T + fc_b       # [N, V]

    in_maps = []
    for c in range(M):
        sl = slice(c * NL, (c + 1) * NL)
        gctx = cx(gctx_all[sl].T.reshape(3 * H // P, P, NL).astype(np.float32))
        lctx = cx(lctx_all[sl].reshape(NB, P, V).astype(np.float32))
        oh = np.zeros((V, NL), np.float32)
        oh[init_token[sl], np.arange(NL)] = 1.0
        oh0T = cx(oh.reshape(KV, P, NL))
        in_maps.append({
            "whh": whh, "wfold": wfold, "gctx": gctx, "fcwh": fcwh,
            "fcwfold": fcwfold, "lctx": lctx, "bhhn": bhhn, "oh0T": oh0T,
        })

    nc = _get_program()
    res = run_bass_kernel_spmd(nc, in_maps, core_ids=list(range(M)))
    LAST_RESULT = res
    out = np.empty((N, T_STEPS, V), np.float32)
    for c in range(M):
        out[c * NL:(c + 1) * NL] = res.results[c]["out"]
    return out


# revision 26
# speedup vs baseline: 1.1772x; 1.1772x over previous
import sys

sys.path.insert(0, "/opt/trn_rl_repo")

import numpy as np

import concourse.bass as bass
import concourse.mybir as mybir
import concourse.tile as tile
from concourse import bacc
from concourse.bass_utils import run_bass_kernel_spmd
from concourse.masks import make_identity

# Problem dims (hardcoded per harness contract)
N, S, C = 4096, 1, 512
E, H, V = 64, 512, 256
T_STEPS = 32
M = 8            # cores
NL = N // M      # 512 rows per core
P = 128
KH = H // P      # 4 k-tiles over hidden dim
KV = V // P      # 2 k-tiles over vocab dim
NB = NL // P     # 4 batch tiles per core

F32 = mybir.dt.float32
F32R = mybir.dt.float32r
BF16 = mybir.dt.bfloat16

_PROGRAM = None
LAST_RESULT = None


def _build_program():
    nc = bacc.Bacc("TRN2", target_bir_lowering=False, debug=False)

    whh_d = nc.dram_tensor("whh", [KH, P, 3 * H], F32R, kind="ExternalInput")
    wfold_d = nc.dram_tensor("wfold", [KV, P, 3 * H], F32R, kind="ExternalInput")
    gctx_d = nc.dram_tensor("gctx", [3 * H // P, P, NL], F32, kind="ExternalInput")
    fcwh_d = nc.dram_tensor("fcwh", [KH, P, V], F32R, kind="ExternalInput")
    fcwfold_d = nc.dram_tensor("fcwfold", [KV, P, V], F32R, kind="ExternalInput")
    lctx_d = nc.dram_tensor("lctx", [NB, P, V], F32, kind="ExternalInput")
    bhhn_d = nc.dram_tensor("bhhn", [P, KH], F32, kind="ExternalInput")
    oh0T_d = nc.dram_tensor("oh0T", [KV, P, NL], F32R, kind="ExternalInput")
    out_d = nc.dram_tensor("out", [NL, T_STEPS, V], F32, kind="ExternalOutput")

    Sig = mybir.ActivationFunctionType.Sigmoid
    Copy = mybir.ActivationFunctionType.Copy
    Tanh = mybir.ActivationFunctionType.Tanh
    ADD = mybir.AluOpType.add
    MULT = mybir.AluOpType.mult
    SUB = mybir.AluOpType.subtract
    ISEQ = mybir.AluOpType.is_equal
    MG = 3 * H // P  # 12 gate m-tiles

    with tile.TileContext(nc) as tc:
        with tc.tile_pool(name="const", bufs=1) as const, \
             tc.tile_pool(name="state", bufs=2) as state, \
             tc.tile_pool(name="gate", bufs=2) as gate, \
             tc.tile_pool(name="work", bufs=4) as work, \
             tc.tile_pool(name="outp", bufs=2) as outp, \
             tc.tile_pool(name="pg", bufs=3, space="PSUM") as pgp, \
             tc.tile_pool(name="px", bufs=2, space="PSUM") as pxp, \
             tc.tile_pool(name="pl", bufs=2, space="PSUM") as plp, \
             tc.tile_pool(name="pt", bufs=1, space="PSUM") as ptp:

            identb = const.tile([P, P], BF16)
            make_identity(nc, identb)

            # ---- constant loads (ordered so step-0 deps come first) ----
            oh0T = const.tile([P, KV, NL], F32R)
            for k in range(KV):
                nc.sync.dma_start(out=oh0T[:, k, :], in_=oh0T_d[k])
            wfold = const.tile([P, KV, 3 * H], F32R)
            for k in range(KV):
                nc.sync.dma_start(out=wfold[:, k, :], in_=wfold_d[k])
            gctx = const.tile([P, MG, NL], F32)
            for k in range(MG):
                nc.sync.dma_start(out=gctx[:, k, :], in_=gctx_d[k])
            bhhn = const.tile([P, KH], F32)
            nc.sync.dma_start(out=bhhn, in_=bhhn_d[:, :])
            fcwh = const.tile([P, KH, V], F32R)
            for k in range(KH):
                nc.sync.dma_start(out=fcwh[:, k, :], in_=fcwh_d[k])
            fcwfold = const.tile([P, KV, V], F32R)
            for k in range(KV):
                nc.sync.dma_start(out=fcwfold[:, k, :], in_=fcwfold_d[k])
            lctx = const.tile([P, NB, V], F32)
            for k in range(NB):
                nc.sync.dma_start(out=lctx[:, k, :], in_=lctx_d[k])
            whh = const.tile([P, KH, 3 * H], F32R)
            for k in range(KH):
                nc.sync.dma_start(out=whh[:, k, :], in_=whh_d[k])

            HF = NL // 2
            hT_prev = None
            oh_prev = None
            for t in range(T_STEPS):
                r_t = gate.tile([P, KH, NL], F32, tag="r")
                z_t = gate.tile([P, KH, NL], F32, tag="z")
                n_t = gate.tile([P, KH, NL], F32, tag="n")
                hT_cur = state.tile([P, KH, NL], F32R, tag="h")
                oht = oh0T if t == 0 else state.tile([P, KV, NL], F32R,
                                                     tag="oht")
                mx = work.tile([P, NB], F32, tag="mx")
                oh_nv = gate.tile([P, NB, V], BF16, tag="oh")

                for q in range(2):
                    qsl = slice(q * HF, (q + 1) * HF)
                    # ---- one-hot transpose for this half (prev step) ----
                    if t > 0:
                        pt = ptp.tile([P, NL], BF16, tag="pt")
                        for vb in range(KV):
                            for nb in range(2):
                                nc.tensor.transpose(
                                    pt[:, vb * HF + nb * P:vb * HF + (nb + 1) * P],
                                    oh_prev[:, nb + 2 * q, vb * P:(vb + 1) * P],
                                    identb)
                        for vb in range(KV):
                            nc.vector.tensor_copy(
                                oht[:, vb, qsl],
                                pt[:, vb * HF:(vb + 1) * HF])

                    # ---- r,z gates for this half ----
                    for m in range(2 * KH):
                        msl = slice(m * P, (m + 1) * P)
                        pg = pgp.tile([P, NL], F32, tag="pg")
                        pgv = pg[:, 0:HF]
                        dst = (r_t[:, m, qsl] if m < KH
                               else z_t[:, m - KH, qsl])
                        if t > 0:
                            nc.scalar.activation(pgv, gctx[:, m, qsl], Copy,
                                                 0.0, 1.0)
                            for k in range(KH):
                                nc.tensor.matmul(pgv, whh[:, k, msl],
                                                 hT_prev[:, k, qsl],
                                                 start=False, stop=False,
                                                 skip_group_check=True)
                            nc.tensor.matmul(pgv, wfold[:, 0, msl],
                                             oht[:, 0, qsl],
                                             start=False, stop=False,
                                             skip_group_check=True)
                            nc.tensor.matmul(pgv, wfold[:, 1, msl],
                                             oht[:, 1, qsl],
                                             start=False, stop=True,
                                             skip_group_check=True)
                            nc.scalar.activation(dst, pgv, Sig)
                        else:
                            nc.tensor.matmul(pgv, wfold[:, 0, msl],
                                             oht[:, 0, qsl],
                                             start=True, stop=False)
                            nc.tensor.matmul(pgv, wfold[:, 1, msl],
                                             oht[:, 1, qsl],
                                             start=False, stop=True)
                            nc.vector.tensor_add(dst, pgv, gctx[:, m, qsl])
                            nc.scalar.activation(dst, dst, Sig)

                    # ---- n gate + h update for this half ----
                    pxs = []
                    for i in range(KH):
                        m = 2 * KH + i
                        msl = slice(m * P, (m + 1) * P)
                        px = pxp.tile([P, NL], F32, tag="px")
                        pxv = px[:, 0:HF]
                        pxs.append(pxv)
                        if t > 0:
                            nc.vector.tensor_copy(pxv, gctx[:, m, qsl])
                            nc.tensor.matmul(pxv, wfold[:, 0, msl],
                                             oht[:, 0, qsl],
                                             start=False, stop=False,
                                             skip_group_check=True)
                            nc.tensor.matmul(pxv, wfold[:, 1, msl],
                                             oht[:, 1, qsl],
                                             start=False, stop=True,
                                             skip_group_check=True)
                        else:
                            nc.tensor.matmul(pxv, wfold[:, 0, msl],
                                             oht[:, 0, qsl],
                                             start=True, stop=False)
                            nc.tensor.matmul(pxv, wfold[:, 1, msl],
                                             oht[:, 1, qsl],
                                             start=False, stop=True)
                    for i in range(KH):
                        m = 2 * KH + i
                        msl = slice(m * P, (m + 1) * P)
                        u = work.tile([P, NL], F32, tag="u")
                        uv = u[:, 0:HF]
                        if t > 0:
                            pgh = pgp.tile([P, NL], F32, tag="pg")
                            pghv = pgh[:, 0:HF]
                            for k in range(KH):
                                nc.tensor.matmul(pghv, whh[:, k, msl],
                                                 hT_prev[:, k, qsl],
                                                 start=(k == 0),
                                                 stop=(k == KH - 1))
                            nc.vector.scalar_tensor_tensor(
                                uv, pghv, bhhn[:, i:i + 1], r_t[:, i, qsl],
                                ADD, MULT)
                        else:
                            nc.vector.tensor_scalar(uv, r_t[:, i, qsl],
                                                    bhhn[:, i:i + 1],
                                                    None, MULT)
                        nc.vector.tensor_add(n_t[:, i, qsl], pxs[i], uv)
                        if t == 0:
                            nc.vector.tensor_add(n_t[:, i, qsl],
                                                 n_t[:, i, qsl],
                                                 gctx[:, m, qsl])
                        nc.scalar.activation(n_t[:, i, qsl], n_t[:, i, qsl],
                                             Tanh)
                        v = work.tile([P, NL], F32, tag="v")
                        vv = v[:, 0:HF]
                        if t > 0:
                            nc.gpsimd.tensor_sub(vv, hT_prev[:, i, qsl],
                                                 n_t[:, i, qsl])
                            nc.gpsimd.tensor_mul(vv, vv, z_t[:, i, qsl])
                            nc.vector.tensor_add(hT_cur[:, i, qsl], vv,
                                                 n_t[:, i, qsl])
                        else:
                            nc.vector.tensor_scalar(vv, z_t[:, i, qsl],
                                                    -1.0, 1.0, MULT, ADD)
                            nc.vector.tensor_mul(hT_cur[:, i, qsl], vv,
                                                 n_t[:, i, qsl])

                    # ---- logits + argmax for this half ----
                    for nb in (2 * q, 2 * q + 1):
                        nsl = slice(nb * P, (nb + 1) * P)
                        pl = plp.tile([P, NL], F32, tag="pl")
                        plv = pl[:, 0:V]
                        if t > 0:
                            nc.scalar.activation(plv, lctx[:, nb, :], Copy,
                                                 0.0, 1.0)
                            nc.tensor.matmul(plv, oht[:, 0, nsl],
                                             fcwfold[:, 0, :],
                                             start=False, stop=False,
                                             skip_group_check=True)
                            nc.tensor.matmul(plv, oht[:, 1, nsl],
                                             fcwfold[:, 1, :],
                                             start=False, stop=False,
                                             skip_group_check=True)
                            for k in range(KH):
                                nc.tensor.matmul(plv, hT_cur[:, k, nsl],
                                                 fcwh[:, k, :],
                                                 start=False,
                                                 stop=(k == KH - 1),
                                                 skip_group_check=True)
                            lg = outp.tile([P, V], F32, tag="lg")
                            nc.scalar.activation(lg, plv, Copy, 0.0, 1.0)
                        else:
                            nc.tensor.matmul(plv, oht[:, 0, nsl],
                                             fcwfold[:, 0, :],
                                             start=True, stop=False)
                            nc.tensor.matmul(plv, oht[:, 1, nsl],
                                             fcwfold[:, 1, :],
                                             start=False, stop=False)
                            for k in range(KH):
                                nc.tensor.matmul(plv, hT_cur[:, k, nsl],
                                                 fcwh[:, k, :],
                                                 start=False,
                                                 stop=(k == KH - 1))
                            lg = outp.tile([P, V], F32, tag="lg")
                            nc.vector.tensor_add(lg, plv, lctx[:, nb, :])
                        nc.sync.dma_start(out=out_d[nsl, t, :], in_=lg)
                        if t < T_STEPS - 1:
                            amsrc = plv if t > 0 else lg
                            nc.vector.tensor_reduce(out=mx[:, nb:nb + 1],
                                                    in_=amsrc,
                                                    axis=mybir.AxisListType.X,
                                                    op=mybir.AluOpType.max)
                            nc.vector.tensor_scalar(oh_nv[:, nb, :], amsrc,
                                                    mx[:, nb:nb + 1], None,
                                                    ISEQ)

                oh_prev = oh_nv
                hT_prev = hT_cur

    nc.compile()
    return nc


def _get_program():
    global _PROGRAM
    if _PROGRAM is None:
        _PROGRAM = _build_program()
    return _PROGRAM


def kernel(encoded, init_token, emb_W, W_ih, W_hh, b_ih, b_hh, fc_W, fc_b, T):
    global LAST_RESULT
    assert int(T) == T_STEPS
    encoded = np.asarray(encoded, np.float64)
    init_token = np.asarray(init_token).astype(np.int64)
    emb_W = np.asarray(emb_W, np.float64)
    W_ih = np.asarray(W_ih, np.float64)
    W_hh = np.asarray(W_hh, np.float64)
    b_ih = np.asarray(b_ih, np.float64)
    b_hh = np.asarray(b_hh, np.float64)
    fc_W = np.asarray(fc_W, np.float64)
    fc_b = np.asarray(fc_b, np.float64)

    cx = np.ascontiguousarray

    # shared weights
    whh = cx(W_hh.T.reshape(KH, P, 3 * H).astype(np.float32))
    wfold = cx((W_ih[:, :E] @ emb_W.T).T.reshape(KV, P, 3 * H).astype(np.float32))
    fcwh = cx(fc_W[:, E + C:].T.reshape(KH, P, V).astype(np.float32))
    fcwfold = cx((fc_W[:, :E] @ emb_W.T).T.reshape(KV, P, V).astype(np.float32))
    bhhn = cx(b_hh[2 * H:].reshape(KH, P).T.astype(np.float32))

    # context GEMMs precomputed exactly on host (fp64)
    ctx_all = encoded.reshape(N, C)
    bias_g = b_ih.copy()
    bias_g[:2 * H] += b_hh[:2 * H]
    gctx_all = ctx_all @ W_ih[:, E:].T + bias_g          # [N, 3H]
    lctx_all = ctx_all @ fc_W[:, E:E + C].T + fc_b       # [N, V]

    in_maps = []
    for c in range(M):
        sl = slice(c * NL, (c + 1) * NL)
        gctx = cx(gctx_all[sl].T.reshape(3 * H // P, P, NL).astype(np.float32))
        lctx = cx(lctx_all[sl].reshape(NB, P, V).astype(np.float32))
        oh = np.zeros((V, NL), np.float32)
        oh[init_token[sl], np.arange(NL)] = 1.0
        oh0T = cx(oh.reshape(KV, P, NL))
        in_maps.append({
            "whh": whh, "wfold": wfold, "gctx": gctx, "fcwh": fcwh,
            "fcwfold": fcwfold, "lctx": lctx, "bhhn": bhhn, "oh0T": oh0T,
        })

    nc = _get_program()
    res = run_bass_kernel_spmd(nc, in_maps, core_ids=list(range(M)))
    LAST_RESULT = res
    out = np.empty((N, T_STEPS, V), np.float32)
    for c in range(M):
        out[c * NL:(c + 1) * NL] = res.results[c]["out"]
    return out
